# revision 1
# baseline (speedup 1.0000x reference)
"""Trainium2 Bass kernel for grouped vector attention (sparse_attention).

Reference computation (B=2, L1=L2=512, D=256, g=16, n=16):
    Q = x_target @ Wq.T ; K = x_source @ Wk.T ; V = x_source @ Wv.T
    diff = Q.reshape(B,L1,1,n,g) - K.reshape(B,1,L2,n,g)
    scores = relu(einsum('bijng,g->bijn', relu(diff), w_mlp) + b_mlp)
    att = softmax(scores, axis=2)                      # over L2
    out = einsum('bijn,bjgn->bign', att, V.reshape(B,L2,g,n)).reshape(B,L1,D)

Sharding: 8 cores = 2 batches x 4 L2(j)-quarters. Each core handles all 512
queries against its 128 source positions and produces partial (unnormalized)
outputs + partial softmax denominators; the host sums the 4 partials per
batch and divides. Sharding over j (not i) means the exp'd scores come out
with j on partitions — exactly what the att@V contraction needs, so there is
no on-chip transpose anywhere.

Per-core pipeline, for each quad of 4 source positions (32 quads):
  - tmp[d, i] = relu(Q[i,d] - K[j,d]) with d on partitions, i free:
      ScalarE:  activation(Relu, in=QT, bias=-K[:,j], scale=1)
      VectorE:  tensor_scalar(in=QT, s1=-K[:,j], s2=0, op0=add, op1=max)
  - grouped weighted sum over g=16 via TensorE matmul with block-diagonal
    [128 x 32] sel (w_mlp folded); j's 16 scores land in PSUM slot 32*jj.
  - p = exp(scores + b) off PSUM; pc = max(p, 1)   (= exp(relu(scores+b)))
  - V_sel[g][32*jj + nn, e] = V[4g+jj, e] * (e % 16 == nn)  (built once per
    quad-pair by broadcast-DMA from a DRAM copy of V + one masked multiply)
  - out_partial[e, i]  += V_sel[g][:, e-half].T @ pc   (PSUM accumulation
    across all 32 quads);  S_partial[nn, i] += ones_sel.T @ pc
"""

import numpy as np

import concourse.bass as bass
import concourse.bacc as bacc
import concourse.tile as tile
import concourse.mybir as mybir
from concourse.bass_utils import run_bass_kernel_spmd

import ml_dtypes

F32 = mybir.dt.float32
BF16 = mybir.dt.bfloat16
AL = mybir.AluOpType
AF = mybir.ActivationFunctionType

B, L1, L2, D = 2, 512, 512, 256
G = 16           # group size (d_group)
N = 16           # number of groups
NCORES = 8
JSH = 128        # source positions per core (L2 / 4)
NQUAD = 32       # 32 quads of 4 source positions
BF = ml_dtypes.bfloat16

# elementwise engine rotation per (j,h) unit: 0=VectorE, 1=ScalarE, 2=GpSimd
# (GpSimd tensor_scalar measured 7.5us/op on HW - never assign it)
ENGINE_PATTERN = (
    0, 1, 0, 0, 1, 0, 1, 0, 0, 1, 0, 0, 1, 0, 1, 0,
    0, 1, 0, 0, 1, 0, 1, 0, 0, 1, 0, 0, 1, 0, 0, 0,
)
GPS_TT = False  # GpSimd streaming degrades VectorE via the shared SBUF port


def _build(b_val: float):
    """Build + compile the per-core Bass graph. Same graph for all 8 cores."""
    nc = bacc.Bacc(
        "TRN2", target_bir_lowering=False, debug=False, enable_asserts=False
    )

    # ---- DRAM parameters (per-core shards, host-prepped) ----
    xtT_d = nc.dram_tensor("xtT", [2, 128, L1], BF16, kind="ExternalInput")
    xssT_d = nc.dram_tensor("xssT", [2, 128, JSH], BF16, kind="ExternalInput")
    wqT_d = nc.dram_tensor("wqT", [2, 128, D], BF16, kind="ExternalInput")
    wkT_d = nc.dram_tensor("wkT", [2, 128, D], BF16, kind="ExternalInput")
    wvT_d = nc.dram_tensor("wvT", [2, 128, D], BF16, kind="ExternalInput")
    sel_d = nc.dram_tensor("sel", [2, 128, 32], BF16, kind="ExternalInput")
    vmask_d = nc.dram_tensor("vmask", [128, 2 * D], BF16, kind="ExternalInput")
    ones_d = nc.dram_tensor("ones_sel", [128, N], BF16, kind="ExternalInput")
    outp_d = nc.dram_tensor("outp", [2, 128, L1], F32, kind="ExternalOutput")
    souts_d = nc.dram_tensor("souts", [N, L1], F32, kind="ExternalOutput")
    vdram = nc.dram_tensor("vdram", [JSH, D], BF16)

    with tile.TileContext(nc) as tc:
        with (
            tc.tile_pool(name="const", bufs=1) as cpool,
            tc.tile_pool(name="vselp", bufs=1) as vpool,
            tc.tile_pool(name="work", bufs=4) as wpool,
            tc.tile_pool(name="tmps", bufs=12) as tpool,
            tc.tile_pool(name="ps_s", bufs=2, space="PSUM") as ps_pool,
            tc.tile_pool(name="ps_acc", bufs=1, space="PSUM") as pa_pool,
        ):
            # ---- load constants / inputs ----
            xtT = [cpool.tile([128, L1], BF16, name=f"xtT{h}") for h in range(2)]
            xssT = [cpool.tile([128, JSH], BF16, name=f"xssT{h}") for h in range(2)]
            wqT = [cpool.tile([128, D], BF16, name=f"wqT{h}") for h in range(2)]
            wkT = [cpool.tile([128, D], BF16, name=f"wkT{h}") for h in range(2)]
            wvT = [cpool.tile([128, D], BF16, name=f"wvT{h}") for h in range(2)]
            sel = [cpool.tile([128, 32], BF16, name=f"sel{h}") for h in range(2)]
            vmask = cpool.tile([128, 2 * D], BF16, name="vmask")
            ones_sel = cpool.tile([128, N], BF16, name="ones_sel")
            bml = cpool.tile([128, 1], F32, name="bml")
            nc.vector.memset(bml[:], float(b_val))

            # ---- accumulators (also the warm-up target: quad 0's V-matmul
            # uses start=True, which clears whatever the warm-up wrote) ----
            ops = [
                pa_pool.tile([128, L1], F32, name=f"ops{eh}") for eh in range(2)
            ]
            sps = pa_pool.tile([16, L1], F32, name="sps")

            # ---- PE warm-up burst: self-contained (memset inputs), runs at
            # t~0 so HAM flips to 8/8 and stays there until real matmuls flow
            wz = cpool.tile([128, L1], BF16, name="wz")
            nc.vector.memset(wz[:], 0.25)
            for k in range(12):
                nc.tensor.matmul(
                    ops[0][0:32, 0:256],
                    wz[:, 0:32],
                    wz[:, 0:256],
                    start=(k == 0),
                    stop=(k == 11),
                    skip_group_check=True,
                )
            for h in range(2):
                nc.sync.dma_start(xssT[h][:], xssT_d[h])
                nc.sync.dma_start(wvT[h][:], wvT_d[h])
                nc.sync.dma_start(sel[h][:], sel_d[h])
                nc.sync.dma_start(xtT[h][:], xtT_d[h])
                nc.scalar.dma_start(wqT[h][:], wqT_d[h])
                nc.scalar.dma_start(wkT[h][:], wkT_d[h])
            nc.scalar.dma_start(vmask[:], vmask_d[:])
            nc.scalar.dma_start(ones_sel[:], ones_d[:])

            # ---- V first: projection -> DRAM -> per-quad broadcast+mask ----
            Vt = cpool.tile([128, D], BF16, name="Vt")
            psv = ps_pool.tile([128, D], F32, name="psv", tag="psk", bufs=1)
            for dh in range(2):
                nc.tensor.matmul(
                    psv[:],
                    xssT[dh][:],
                    wvT[dh][:],
                    start=(dh == 0),
                    stop=(dh == 1),
                )
            nc.scalar.copy(Vt[:], psv[:])
            nc.sync.dma_start(vdram[:], Vt[:])

            V_sel = [
                vpool.tile([128, 2 * D], BF16, name=f"vs{k}")
                for k in range(NQUAD // 2)
            ]

            def build_pair(k):
                vs2 = V_sel[k]
                for half in range(2):
                    gq = 2 * k + half
                    bsrc = (
                        vdram.ap()[4 * gq : 4 * gq + 4, :]
                        .unsqueeze(1)
                        .broadcast_to((4, 32, D))
                    )
                    nc.sync.dma_start(vs2[:, half * D : (half + 1) * D], bsrc)
                nc.vector.tensor_tensor(vs2[:], vs2[:], vmask[:], op=AL.mult)

            for k in range(6):
                build_pair(k)


            # ---- projections on device ----
            # QT[h] (128 e, 512 i) bf16; KTn[h] (128 e, 128 j) f32 negated.
            # eh=0 pair first so h=0 elementwise units can start early.
            QT = [cpool.tile([128, L1], BF16, name=f"QT{h}") for h in range(2)]
            KTn = [cpool.tile([128, JSH], F32, name=f"KTn{h}") for h in range(2)]
            for eh in range(2):
                psk = ps_pool.tile([128, JSH], F32, name="psk", tag="psk", bufs=1)
                for dh in range(2):
                    nc.tensor.matmul(
                        psk[:],
                        wkT[dh][:, eh * 128 : (eh + 1) * 128],
                        xssT[dh][:],
                        start=(dh == 0),
                        stop=(dh == 1),
                    )
                nc.scalar.mul(KTn[eh][:], psk[:], -1.0)
                psq = ps_pool.tile([128, L1], F32, name="psq", tag="ps_s")
                for dh in range(2):
                    nc.tensor.matmul(
                        psq[:],
                        wqT[dh][:, eh * 128 : (eh + 1) * 128],
                        xtT[dh][:],
                        start=(dh == 0),
                        stop=(dh == 1),
                    )
                nc.scalar.copy(QT[eh][:], psq[:])

            # ---- main loop: 32 quads of 4 source positions ----
            for g in range(NQUAD):
                if g % 2 == 0 and g // 2 + 6 < NQUAD // 2:
                    build_pair(g // 2 + 6)
                ps = ps_pool.tile([128, L1], F32, name="ps", tag="ps_s")
                for jj in range(4):
                    j = 4 * g + jj
                    for h in range(2):
                        u = (g * 4 + jj) * 2 + h
                        eng = ENGINE_PATTERN[u % len(ENGINE_PATTERN)]
                        t = tpool.tile([128, L1], BF16, name="t", tag="t")
                        if eng == 1:
                            # t = relu(Q + (-K))
                            nc.scalar.activation(
                                t[:],
                                QT[h][:],
                                AF.Relu,
                                bias=KTn[h][:, j : j + 1],
                                scale=1.0,
                            )
                        else:
                            # t = max(Q + (-K), 0)
                            nc.vector.tensor_scalar(
                                t[:],
                                QT[h][:],
                                KTn[h][:, j : j + 1],
                                0.0,
                                AL.add,
                                AL.max,
                            )
                        nc.tensor.matmul(
                            ps[32 * jj : 32 * jj + 32, :],
                            sel[h][:],
                            t[:],
                            start=(h == 0),
                            stop=(h == 1),
                            tile_position=(0, 32 * jj),
                        )
                # p = exp(scores + b); pc = max(p, 1) = exp(relu(scores + b))
                p = wpool.tile([128, L1], BF16, name="p", tag="p", bufs=6)
                nc.scalar.activation(p[:], ps[:], AF.Exp, bias=bml[:], scale=1.0)
                pc = wpool.tile([128, L1], BF16, name="pc", tag="pc", bufs=16)
                nc.vector.tensor_scalar(pc[:], p[:], 1.0, None, AL.max)
                # accumulate partial outputs and denominators
                for eh in range(2):
                    off = (g % 2) * D + eh * 128
                    nc.tensor.matmul(
                        ops[eh][:],
                        V_sel[g // 2][:, off : off + 128],
                        pc[:],
                        start=(g == 0),
                        stop=(g == NQUAD - 1),
                        skip_group_check=True,
                    )
                nc.tensor.matmul(
                    sps[:],
                    ones_sel[:, 0:N],
                    pc[:],
                    start=(g == 0),
                    stop=(g == NQUAD - 1),
                    skip_group_check=True,
                )

            # ---- evacuate + store ----
            for eh in range(2):
                ou = wpool.tile([128, L1], F32, name="ou", tag="ou", bufs=2)
                ceng = nc.vector if eh == 0 else nc.scalar
                if eh == 0:
                    nc.vector.tensor_copy(ou[:], ops[eh][:])
                else:
                    nc.scalar.copy(ou[:], ops[eh][:])
                nc.sync.dma_start(outp_d[eh], ou[:])
            so = wpool.tile([16, L1], F32, name="so")
            nc.scalar.copy(so[:], sps[:])
            nc.sync.dma_start(souts_d[:], so[:])

    nc.compile()
    return nc


_CACHE: dict = {}


def _get_graph(b_val: float):
    key = round(float(b_val), 10)
    if key not in _CACHE:
        _CACHE[key] = _build(float(b_val))
    return _CACHE[key]


def _host_prep(x_source, x_target, Wq, Wk, Wv, w_mlp):
    """Build per-core input maps (numpy, bf16)."""
    w_full = np.tile(np.asarray(w_mlp, np.float32), D // G)  # w_full[d] = w[d%16]
    sel = np.zeros((2, 128, 32), np.float32)
    for h in range(2):
        for dl in range(128):
            d = 128 * h + dl
            sel[h, dl, d // G] = w_full[d]
    # V_sel mask: row p = 32*jj + s (s<16 valid), col e: keep if e%16 == s
    vmask = np.zeros((128, 2 * D), np.float32)
    for p in range(128):
        s = p % 32
        if s < 16:
            vmask[p, s::G] = 1.0
    # S selector: row p = 32*jj + s -> column s (s < 16)
    ones_sel = np.zeros((128, N), np.float32)
    for p in range(128):
        s = p % 32
        if s < 16:
            ones_sel[p, s] = 1.0

    def split_h(a):  # (256, X) -> (2, 128, X)
        return np.ascontiguousarray(a.reshape(2, 128, a.shape[1]))

    wq_b = split_h(np.asarray(Wq, np.float32).T).astype(BF)
    wk_b = split_h(np.asarray(Wk, np.float32).T).astype(BF)
    wv_b = split_h(np.asarray(Wv, np.float32).T).astype(BF)
    sel_b = sel.astype(BF)
    vmask_b = vmask.astype(BF)
    ones_b = ones_sel.astype(BF)

    xtT = [
        split_h(np.asarray(x_target[b], np.float32).T).astype(BF) for b in range(B)
    ]
    xsT = [np.asarray(x_source[b], np.float32).T for b in range(B)]
    in_maps = []
    for core in range(NCORES):
        b, jq = divmod(core, 4)
        j0 = jq * JSH
        xssT = split_h(xsT[b][:, j0 : j0 + JSH]).astype(BF)
        in_maps.append(
            {
                "xtT": xtT[b],
                "xssT": xssT,
                "wqT": wq_b,
                "wkT": wk_b,
                "wvT": wv_b,
                "sel": sel_b,
                "vmask": vmask_b,
                "ones_sel": ones_b,
            }
        )
    return in_maps


def _host_gather(results):
    """Sum partials over j-shards, normalize, reshape to (B, L1, D)."""
    out = np.empty((B, L1, D), np.float32)
    for b in range(B):
        cores = [b * 4 + jq for jq in range(4)]
        U = sum(
            results[c]["outp"].reshape(D, L1).astype(np.float64) for c in cores
        )  # (e, i)
        S = sum(results[c]["souts"].astype(np.float64) for c in cores)  # (nn, i)
        att = U / S[np.arange(D) % N, :]  # (e, i)
        out[b] = att.T.astype(np.float32)
    return out


def run(inputs, trace=False, **kwargs):
    nc = _get_graph(float(np.asarray(inputs["b_mlp"]).reshape(-1)[0]))
    in_maps = _host_prep(
        inputs["x_source"],
        inputs["x_target"],
        inputs["Wq"],
        inputs["Wk"],
        inputs["Wv"],
        inputs["w_mlp"],
    )
    res = run_bass_kernel_spmd(
        nc, in_maps, core_ids=list(range(NCORES)), trace=trace, **kwargs
    )
    return _host_gather(res.results), res


def kernel(**inputs) -> np.ndarray:
    out, _ = run(inputs, trace=False)
    return out



# revision 7
# speedup vs baseline: 1.0027x; 1.0027x over previous
"""Trainium2 Bass kernel for grouped vector attention (sparse_attention).

Reference computation (B=2, L1=L2=512, D=256, g=16, n=16):
    Q = x_target @ Wq.T ; K = x_source @ Wk.T ; V = x_source @ Wv.T
    diff = Q.reshape(B,L1,1,n,g) - K.reshape(B,1,L2,n,g)
    scores = relu(einsum('bijng,g->bijn', relu(diff), w_mlp) + b_mlp)
    att = softmax(scores, axis=2)                      # over L2
    out = einsum('bijn,bjgn->bign', att, V.reshape(B,L2,g,n)).reshape(B,L1,D)

Sharding: 8 cores = 2 batches x 4 L2(j)-quarters. Each core handles all 512
queries against its 128 source positions and produces partial (unnormalized)
outputs + partial softmax denominators; the host sums the 4 partials per
batch and divides. Sharding over j (not i) means the exp'd scores come out
with j on partitions — exactly what the att@V contraction needs, so there is
no on-chip transpose anywhere.

Per-core pipeline, per source position j (128 of them):
  - t[d, (h,i)] = relu(8*Q[i,d] - 8*K[j,d]) in fp8e4m3, both 128-d halves
    packed side by side in one [128, 2, 512] tile:
      ScalarE:  activation(Relu, in=QT[h], bias=-8K[:,j], scale=1)
      VectorE:  tensor_scalar(in=QT[h], s1=-8K[:,j], s2=0, op0=add, op1=max)
    (Wq/Wk pre-scaled x8 on host so fp8 values land in e4m3 normal range.)
  - grouped weighted sum over g=16 via ONE fp8 DoubleRow matmul per j:
    lhsT = sel [128, 2, 32] (2*w folded, block-diagonal), rhs = t [128,2,512],
    contracting all 256 d at 0.5 cyc/row; j's 16 scores land at PSUM row
    32*jj. Scores come out scaled x16; exp applies scale=1/16.
  - p = exp(scores/16 + b) off PSUM; pc = max(p, 1)  (= exp(relu(..)))
  - V path stays bf16: V_sel[g][32*jj + nn, e] = V[4g+jj, e] * (e%16 == nn)
    built per quad-pair by broadcast-DMA from a DRAM copy of V + masked mult;
    out_partial[e,i] += V_sel.T @ pc ; S_partial[nn,i] += ones_sel.T @ pc
    (PSUM accumulation across all 32 quads).
  - exp/pc/V-matmul issue is software-pipelined one quad behind the
    elementwise+score stream so no engine queue head-of-line blocks.
"""

import numpy as np

import concourse.bass as bass
import concourse.bacc as bacc
import concourse.tile as tile
import concourse.mybir as mybir
from concourse.bass_utils import run_bass_kernel_spmd

import ml_dtypes

F32 = mybir.dt.float32
BF16 = mybir.dt.bfloat16
FP8 = mybir.dt.float8e4
AL = mybir.AluOpType
AF = mybir.ActivationFunctionType
DR = mybir.MatmulPerfMode.DoubleRow

B, L1, L2, D = 2, 512, 512, 256
G = 16           # group size (d_group)
N = 16           # number of groups
NCORES = 8
JSH = 128        # source positions per core (L2 / 4)
NQUAD = 32       # 32 quads of 4 source positions
BF = ml_dtypes.bfloat16
F8 = ml_dtypes.float8_e4m3

SCALE_T = 8.0    # folded into Wq/Wk on host: t = relu(8q-8k) in e4m3 range
SEL_SCALE = 2.0  # sel = 2*w so fp8 weights stay in normal range
EXP_SCALE = 1.0 / (SCALE_T * SEL_SCALE)

# elementwise engine per (j,h) unit: 0=VectorE, 1=ScalarE (GpSimd measured
# 7.5us/op on HW - never assign it). 5 Scalar per 16 units (~76/256 total):
# VectorE does a unit in ~345ns, ScalarE in ~715ns + the 32 exps.
ENGINE_PATTERN = (
    0, 1, 0, 0, 0, 1, 0, 0,
    0, 1, 0, 0, 1, 0, 0, 1,
)


def _build(b_val: float):
    """Build + compile the per-core Bass graph. Same graph for all 8 cores."""
    nc = bacc.Bacc(
        "TRN2", target_bir_lowering=False, debug=False, enable_asserts=False
    )

    # ---- DRAM parameters (per-core shards, host-prepped) ----
    xtT_d = nc.dram_tensor("xtT", [2, 128, L1], BF16, kind="ExternalInput")
    xssT_d = nc.dram_tensor("xssT", [2, 128, JSH], BF16, kind="ExternalInput")
    wqT_d = nc.dram_tensor("wqT", [2, 128, D], BF16, kind="ExternalInput")
    wkT_d = nc.dram_tensor("wkT", [2, 128, D], BF16, kind="ExternalInput")
    wvT_d = nc.dram_tensor("wvT", [2, 128, D], BF16, kind="ExternalInput")
    # walrus rejects DoubleRow + tile_position col offsets, so each jj slot
    # gets its own 128-wide sel variant (nonzero only in cols 32jj..32jj+15)
    # and the quad's 4 matmuls accumulate into one PSUM tile.
    sel_d = nc.dram_tensor("sel", [4, 128, 2, 128], FP8, kind="ExternalInput")
    vmask_d = nc.dram_tensor("vmask", [128, 2 * D], BF16, kind="ExternalInput")
    ones_d = nc.dram_tensor("ones_sel", [128, N], BF16, kind="ExternalInput")
    outp_d = nc.dram_tensor("outp", [2, 128, L1], F32, kind="ExternalOutput")
    souts_d = nc.dram_tensor("souts", [N, L1], F32, kind="ExternalOutput")
    vdram = nc.dram_tensor("vdram", [JSH, D], BF16)

    with tile.TileContext(nc) as tc:
        with (
            tc.tile_pool(name="const", bufs=1) as cpool,
            tc.tile_pool(name="vselp", bufs=1) as vpool,
            tc.tile_pool(name="work", bufs=4) as wpool,
            tc.tile_pool(name="tmps", bufs=12) as tpool,
            tc.tile_pool(name="ps_s", bufs=2, space="PSUM") as ps_pool,
            tc.tile_pool(name="ps_acc", bufs=1, space="PSUM") as pa_pool,
        ):
            # ---- load constants / inputs ----
            xtT = [cpool.tile([128, L1], BF16, name=f"xtT{h}") for h in range(2)]
            xssT = [cpool.tile([128, JSH], BF16, name=f"xssT{h}") for h in range(2)]
            wqT = [cpool.tile([128, D], BF16, name=f"wqT{h}") for h in range(2)]
            wkT = [cpool.tile([128, D], BF16, name=f"wkT{h}") for h in range(2)]
            wvT = [cpool.tile([128, D], BF16, name=f"wvT{h}") for h in range(2)]
            sel = [
                cpool.tile([128, 2, 128], FP8, name=f"sel{jj}") for jj in range(4)
            ]
            vmask = cpool.tile([128, 2 * D], BF16, name="vmask")
            ones_sel = cpool.tile([128, N], BF16, name="ones_sel")
            bml = cpool.tile([128, 1], F32, name="bml")
            nc.vector.memset(bml[:], float(b_val))

            # ---- accumulators (also the warm-up target: quad 0's V-matmul
            # uses start=True, which clears whatever the warm-up wrote) ----
            ops = [
                pa_pool.tile([128, L1], F32, name=f"ops{eh}") for eh in range(2)
            ]
            sps = pa_pool.tile([16, L1], F32, name="sps")

            # ---- PE warm-up burst: self-contained (memset inputs), runs at
            # t~0 so HAM flips to 8/8 and stays there until real matmuls flow
            wz = cpool.tile([128, L1], BF16, name="wz")
            nc.vector.memset(wz[:], 0.25)
            for k in range(12):
                nc.tensor.matmul(
                    ops[0][0:32, 0:256],
                    wz[:, 0:32],
                    wz[:, 0:256],
                    start=(k == 0),
                    stop=(k == 11),
                    skip_group_check=True,
                )
            for h in range(2):
                nc.sync.dma_start(xssT[h][:], xssT_d[h])
                nc.sync.dma_start(wvT[h][:], wvT_d[h])
                nc.sync.dma_start(xtT[h][:], xtT_d[h])
                nc.scalar.dma_start(wqT[h][:], wqT_d[h])
                nc.scalar.dma_start(wkT[h][:], wkT_d[h])
            for jj in range(4):
                nc.sync.dma_start(sel[jj][:], sel_d[jj])
            nc.scalar.dma_start(vmask[:], vmask_d[:])
            nc.scalar.dma_start(ones_sel[:], ones_d[:])

            # ---- V first: projection -> DRAM -> per-quad broadcast+mask ----
            Vt = cpool.tile([128, D], BF16, name="Vt")
            psv = ps_pool.tile([128, D], F32, name="psv", tag="psk", bufs=1)
            for dh in range(2):
                nc.tensor.matmul(
                    psv[:],
                    xssT[dh][:],
                    wvT[dh][:],
                    start=(dh == 0),
                    stop=(dh == 1),
                )
            nc.scalar.copy(Vt[:], psv[:])
            nc.sync.dma_start(vdram[:], Vt[:])

            V_sel = [
                vpool.tile([128, 2 * D], BF16, name=f"vs{k}")
                for k in range(NQUAD // 2)
            ]

            def build_pair(k):
                vs2 = V_sel[k]
                for half in range(2):
                    gq = 2 * k + half
                    bsrc = (
                        vdram.ap()[4 * gq : 4 * gq + 4, :]
                        .unsqueeze(1)
                        .broadcast_to((4, 32, D))
                    )
                    nc.sync.dma_start(vs2[:, half * D : (half + 1) * D], bsrc)
                nc.vector.tensor_tensor(vs2[:], vs2[:], vmask[:], op=AL.mult)

            for k in range(6):
                build_pair(k)

            # ---- projections on device ----
            # QT[h] (128 e, 512 i) bf16; KTn[h] (128 e, 128 j) f32 negated.
            # eh=0 pair first so h=0 elementwise units can start early.
            QT = [cpool.tile([128, L1], BF16, name=f"QT{h}") for h in range(2)]
            KTn = [cpool.tile([128, JSH], F32, name=f"KTn{h}") for h in range(2)]
            for eh in range(2):
                psk = ps_pool.tile([128, JSH], F32, name="psk", tag="psk", bufs=1)
                for dh in range(2):
                    nc.tensor.matmul(
                        psk[:],
                        wkT[dh][:, eh * 128 : (eh + 1) * 128],
                        xssT[dh][:],
                        start=(dh == 0),
                        stop=(dh == 1),
                    )
                nc.scalar.mul(KTn[eh][:], psk[:], -1.0)
                psq = ps_pool.tile([128, L1], F32, name="psq", tag="ps_s")
                for dh in range(2):
                    nc.tensor.matmul(
                        psq[:],
                        wqT[dh][:, eh * 128 : (eh + 1) * 128],
                        xtT[dh][:],
                        start=(dh == 0),
                        stop=(dh == 1),
                    )
                nc.scalar.copy(QT[eh][:], psq[:])

            # ---- main loop: 32 quads of 4 source positions, software-
            # pipelined: quad g's exp/pc/V-matmuls issue between quad g+1's
            # elementwise units so no engine queue head-of-line blocks. ----
            state = {}  # g -> dict(ps=..., p=...)

            def issue_units(g, jj):
                j = 4 * g + jj
                t = tpool.tile([128, 2, 512], FP8, name="t", tag="t")
                for h in range(2):
                    u = (g * 8 + jj * 2 + h) % len(ENGINE_PATTERN)
                    eng = ENGINE_PATTERN[u]
                    if eng == 1:
                        # t[:,h,:] = relu(Q + (-8K))
                        nc.scalar.activation(
                            t[:, h, :],
                            QT[h][:],
                            AF.Relu,
                            bias=KTn[h][:, j : j + 1],
                            scale=1.0,
                        )
                    else:
                        # t[:,h,:] = max(Q + (-8K), 0)
                        nc.vector.tensor_scalar(
                            t[:, h, :],
                            QT[h][:],
                            KTn[h][:, j : j + 1],
                            0.0,
                            AL.add,
                            AL.max,
                        )
                ps = state[g]["ps"]
                nc.tensor.matmul(
                    ps[:],
                    sel[jj][:],
                    t[:],
                    start=(jj == 0),
                    stop=(jj == 3),
                    perf_mode=DR,
                )

            def issue_exp(g):
                # p = exp(z/16 + b)
                p = wpool.tile([128, L1], BF16, name="p", tag="p", bufs=6)
                nc.scalar.activation(
                    p[:], state[g]["ps"][:], AF.Exp, bias=bml[:], scale=EXP_SCALE
                )
                state[g]["p"] = p

            def issue_pc(g):
                # pc = max(p, 1) = exp(relu(z/16 + b))
                pc = wpool.tile([128, L1], BF16, name="pc", tag="pc", bufs=16)
                nc.vector.tensor_scalar(pc[:], state[g]["p"][:], 1.0, None, AL.max)
                state[g]["pc"] = pc

            def issue_vmm(g):
                pc = state[g]["pc"]
                for eh in range(2):
                    off = (g % 2) * D + eh * 128
                    nc.tensor.matmul(
                        ops[eh][:],
                        V_sel[g // 2][:, off : off + 128],
                        pc[:],
                        start=(g == 0),
                        stop=(g == NQUAD - 1),
                        skip_group_check=True,
                    )
                nc.tensor.matmul(
                    sps[:],
                    ones_sel[:, 0:N],
                    pc[:],
                    start=(g == 0),
                    stop=(g == NQUAD - 1),
                    skip_group_check=True,
                )
                del state[g]

            for g in range(NQUAD):
                if g % 2 == 0 and g // 2 + 6 < NQUAD // 2:
                    build_pair(g // 2 + 6)
                state[g] = {
                    "ps": ps_pool.tile([128, L1], F32, name="ps", tag="ps_s")
                }
                issue_units(g, 0)
                if g >= 1:
                    issue_exp(g - 1)
                issue_units(g, 1)
                if g >= 1:
                    issue_pc(g - 1)
                issue_units(g, 2)
                issue_units(g, 3)
                if g >= 1:
                    issue_vmm(g - 1)
            issue_exp(NQUAD - 1)
            issue_pc(NQUAD - 1)
            issue_vmm(NQUAD - 1)

            # ---- evacuate + store ----
            for eh in range(2):
                ou = wpool.tile([128, L1], F32, name="ou", tag="ou", bufs=2)
                if eh == 0:
                    nc.vector.tensor_copy(ou[:], ops[eh][:])
                else:
                    nc.scalar.copy(ou[:], ops[eh][:])
                nc.sync.dma_start(outp_d[eh], ou[:])
            so = wpool.tile([16, L1], F32, name="so")
            nc.scalar.copy(so[:], sps[:])
            nc.sync.dma_start(souts_d[:], so[:])

    nc.compile()
    return nc


_CACHE: dict = {}


def _get_graph(b_val: float):
    key = round(float(b_val), 10)
    if key not in _CACHE:
        _CACHE[key] = _build(float(b_val))
    return _CACHE[key]


def _host_prep(x_source, x_target, Wq, Wk, Wv, w_mlp):
    """Build per-core input maps (numpy, bf16/fp8)."""
    w_full = np.tile(np.asarray(w_mlp, np.float32), D // G)  # w_full[d] = w[d%16]
    # fp8 DoubleRow sel: per-jj variant [128 part, 2 h-planes, 128 cols],
    # nonzero only at col 32*jj + group(d)
    sel = np.zeros((4, 128, 2, 128), np.float32)
    for jj in range(4):
        for h in range(2):
            for dl in range(128):
                d = 128 * h + dl
                sel[jj, dl, h, 32 * jj + d // G] = SEL_SCALE * w_full[d]
    # V_sel mask: row p = 32*jj + s (s<16 valid), col e: keep if e%16 == s
    vmask = np.zeros((128, 2 * D), np.float32)
    for p in range(128):
        s = p % 32
        if s < 16:
            vmask[p, s::G] = 1.0
    # S selector: row p = 32*jj + s -> column s (s < 16)
    ones_sel = np.zeros((128, N), np.float32)
    for p in range(128):
        s = p % 32
        if s < 16:
            ones_sel[p, s] = 1.0

    def split_h(a):  # (256, X) -> (2, 128, X)
        return np.ascontiguousarray(a.reshape(2, 128, a.shape[1]))

    wq_b = split_h(SCALE_T * np.asarray(Wq, np.float32).T).astype(BF)
    wk_b = split_h(SCALE_T * np.asarray(Wk, np.float32).T).astype(BF)
    wv_b = split_h(np.asarray(Wv, np.float32).T).astype(BF)
    sel_b = sel.astype(F8)
    vmask_b = vmask.astype(BF)
    ones_b = ones_sel.astype(BF)

    xtT = [
        split_h(np.asarray(x_target[b], np.float32).T).astype(BF) for b in range(B)
    ]
    xsT = [np.asarray(x_source[b], np.float32).T for b in range(B)]
    in_maps = []
    for core in range(NCORES):
        b, jq = divmod(core, 4)
        j0 = jq * JSH
        xssT = split_h(xsT[b][:, j0 : j0 + JSH]).astype(BF)
        in_maps.append(
            {
                "xtT": xtT[b],
                "xssT": xssT,
                "wqT": wq_b,
                "wkT": wk_b,
                "wvT": wv_b,
                "sel": sel_b,
                "vmask": vmask_b,
                "ones_sel": ones_b,
            }
        )
    return in_maps


def _host_gather(results):
    """Sum partials over j-shards, normalize, reshape to (B, L1, D)."""
    out = np.empty((B, L1, D), np.float32)
    for b in range(B):
        cores = [b * 4 + jq for jq in range(4)]
        U = sum(
            results[c]["outp"].reshape(D, L1).astype(np.float64) for c in cores
        )  # (e, i)
        S = sum(results[c]["souts"].astype(np.float64) for c in cores)  # (nn, i)
        att = U / S[np.arange(D) % N, :]  # (e, i)
        out[b] = att.T.astype(np.float32)
    return out


def run(inputs, trace=False, **kwargs):
    nc = _get_graph(float(np.asarray(inputs["b_mlp"]).reshape(-1)[0]))
    in_maps = _host_prep(
        inputs["x_source"],
        inputs["x_target"],
        inputs["Wq"],
        inputs["Wk"],
        inputs["Wv"],
        inputs["w_mlp"],
    )
    res = run_bass_kernel_spmd(
        nc, in_maps, core_ids=list(range(NCORES)), trace=trace, **kwargs
    )
    return _host_gather(res.results), res


def kernel(**inputs) -> np.ndarray:
    out, _ = run(inputs, trace=False)
    return out


# revision 17
# speedup vs baseline: 1.0578x; 1.0549x over previous
"""Trainium2 Bass kernel for grouped vector attention (sparse_attention).

Reference computation (B=2, L1=L2=512, D=256, g=16, n=16):
    Q = x_target @ Wq.T ; K = x_source @ Wk.T ; V = x_source @ Wv.T
    diff = Q.reshape(B,L1,1,n,g) - K.reshape(B,1,L2,n,g)
    scores = relu(einsum('bijng,g->bijn', relu(diff), w_mlp) + b_mlp)
    att = softmax(scores, axis=2)                      # over L2
    out = einsum('bijn,bjgn->bign', att, V.reshape(B,L2,g,n)).reshape(B,L1,D)

Sharding: 8 cores = 2 batches x 4 L2(j)-quarters. Each core handles all 512
queries against its 128 source positions and produces partial (unnormalized)
outputs + partial softmax denominators; the host sums the 4 partials per
batch and divides. Sharding over j (not i) means the exp'd scores come out
with j on partitions — exactly what the att@V contraction needs, so there is
no on-chip transpose anywhere.

Per-core pipeline, per source position j (128 of them):
  - t[d,(h,i)] = relu(8Q[i,d] - 8K[j,d]) with d on partitions (Wq/Wk are
    pre-scaled x8 on host). Each j is assigned to ONE elementwise engine:
      ScalarE j's  -> t in fp8e4m3 (ACT pays no fp8 penalty), scores via ONE
                      fp8 DoubleRow matmul (contracts all 256 d at once,
                      128-wide sel variant per jj slot since walrus rejects
                      DoubleRow + tile_position col offsets)
      VectorE j's  -> t in bf16 (DVE 2x mode; fp8-out measured +105ns), via
                      two bf16 matmuls with 32-col sel at tile_position.
    Scores land x16 scaled in the quad's 32-row PSUM slot.
  - per PAIR of quads: one exp over the [128,1024] 2-bank PSUM span
    (p = exp(z/16 + b)), one pc = max(p,1) on DVE, one paired denominator
    matmul (ones_sel.T @ pc_pair -> [16,1024], host sums the halves).
  - V path bf16: V_sel[g][32*jj+nn, e] = V[4g+jj, e]*(e%16==nn) built per
    quad-pair by broadcast-DMA from a DRAM copy of V + masked mult;
    out_partial[e,i] += V_sel.T @ pc  (PSUM accumulation over all quads).
  - exp/pc/V-matmul issue is software-pipelined one pair behind the
    elementwise+score stream so no engine queue head-of-line blocks.
"""

import numpy as np

import concourse.bass as bass
import concourse.bacc as bacc
import concourse.tile as tile
import concourse.mybir as mybir
from concourse.bass_utils import run_bass_kernel_spmd

import ml_dtypes

F32 = mybir.dt.float32
BF16 = mybir.dt.bfloat16
FP8 = mybir.dt.float8e4
AL = mybir.AluOpType
AF = mybir.ActivationFunctionType
DR = mybir.MatmulPerfMode.DoubleRow

B, L1, L2, D = 2, 512, 512, 256
G = 16           # group size (d_group)
N = 16           # number of groups
NCORES = 8
JSH = 128        # source positions per core (L2 / 4)
NQUAD = 32       # 32 quads of 4 source positions
NPAIR = 16       # pairs of quads
BF = ml_dtypes.bfloat16
F8 = ml_dtypes.float8_e4m3

SCALE_T = 8.0    # folded into Wq/Wk on host: t = relu(8q-8k) in e4m3 range
SEL_SCALE = 2.0  # fp8 sel = 2*w so fp8 weights stay in normal range
EXP_SCALE = 1.0 / (SCALE_T * SEL_SCALE)


def _s_jjs(g):
    """Which jj slots of quad g go down the ScalarE/fp8/DoubleRow path.
    jj=0 always (its DoubleRow matmul starts the quad's PSUM group);
    every 4th quad also jj=1 -> 1.25 S-j's/quad on average, balancing
    ScalarE (694ns/unit + exp) against VectorE (345ns/unit + pc)."""
    return (0, 1) if g % 4 == 3 else (0,)


def _build(b_val: float):
    """Build + compile the per-core Bass graph. Same graph for all 8 cores."""
    nc = bacc.Bacc(
        "TRN2", target_bir_lowering=False, debug=False, enable_asserts=False
    )

    # ---- DRAM parameters (per-core shards, host-prepped) ----
    xtT_d = nc.dram_tensor("xtT", [2, 128, L1], BF16, kind="ExternalInput")
    xssT_d = nc.dram_tensor("xssT", [2, 128, JSH], BF16, kind="ExternalInput")
    wqT_d = nc.dram_tensor("wqT", [2, 128, D], BF16, kind="ExternalInput")
    wkT_d = nc.dram_tensor("wkT", [2, 128, D], BF16, kind="ExternalInput")
    wvT_d = nc.dram_tensor("wvT", [2, 128, D], BF16, kind="ExternalInput")
    self8_d = nc.dram_tensor("sel8", [4, 128, 2, 128], FP8, kind="ExternalInput")
    selb_d = nc.dram_tensor("selb", [2, 128, 32], BF16, kind="ExternalInput")
    vmask_d = nc.dram_tensor("vmask", [128, 2 * D], BF16, kind="ExternalInput")
    ones_d = nc.dram_tensor("ones_sel", [128, N], BF16, kind="ExternalInput")
    outp_d = nc.dram_tensor("outp", [2, 128, L1], F32, kind="ExternalOutput")
    souts_d = nc.dram_tensor("souts", [N, 2, L1], F32, kind="ExternalOutput")
    vdram = nc.dram_tensor("vdram", [JSH, D], BF16)

    with tile.TileContext(nc) as tc:
        with (
            tc.tile_pool(name="const", bufs=1) as cpool,
            tc.tile_pool(name="vselp", bufs=1) as vpool,
            tc.tile_pool(name="work", bufs=4) as wpool,
            tc.tile_pool(name="tmps", bufs=8) as tpool,
            tc.tile_pool(name="ps_s", bufs=2, space="PSUM") as ps_pool,
            tc.tile_pool(name="ps_acc", bufs=1, space="PSUM") as pa_pool,
        ):
            # ---- load constants / inputs, spread over all DMA queues so the
            # K/Q projection inputs land in ~1.5us ----
            xtT = [cpool.tile([128, L1], BF16, name=f"xtT{h}") for h in range(2)]
            xssT = [cpool.tile([128, JSH], BF16, name=f"xssT{h}") for h in range(2)]
            wqT = [cpool.tile([128, D], BF16, name=f"wqT{h}") for h in range(2)]
            wkT = [cpool.tile([128, D], BF16, name=f"wkT{h}") for h in range(2)]
            wvT = [cpool.tile([128, D], BF16, name=f"wvT{h}") for h in range(2)]
            sel8 = [
                cpool.tile([128, 2, 128], FP8, name=f"sel8_{jj}") for jj in range(4)
            ]
            selb = [cpool.tile([128, 32], BF16, name=f"selb{h}") for h in range(2)]
            vmask = cpool.tile([128, 2 * D], BF16, name="vmask")
            ones_sel = cpool.tile([128, N], BF16, name="ones_sel")
            bml = cpool.tile([128, 1], F32, name="bml")
            nc.vector.memset(bml[:], float(b_val))

            # ---- accumulators (also the warm-up target: quad 0's V-matmul
            # uses start=True, which clears whatever the warm-up wrote) ----
            ops = [
                pa_pool.tile([128, L1], F32, name=f"ops{eh}") for eh in range(2)
            ]
            sps = pa_pool.tile([16, 2, L1], F32, name="sps")

            # ---- PE warm-up burst: self-contained (memset inputs), runs at
            # t~0 so HAM flips to 8/8 and stays there until real matmuls flow
            wz = cpool.tile([128, L1], BF16, name="wz")
            nc.vector.memset(wz[:], 0.25)
            for k in range(12):
                nc.tensor.matmul(
                    ops[0][0:32, 0:256],
                    wz[:, 0:32],
                    wz[:, 0:256],
                    start=(k == 0),
                    stop=(k == 11),
                    skip_group_check=True,
                )
            # K-path inputs first (scalar+sync queues), Q-path in parallel
            # (vector+gpsimd), V and the small constants after.
            for h in range(2):
                nc.scalar.dma_start(wkT[h][:], wkT_d[h])
                nc.sync.dma_start(xssT[h][:], xssT_d[h])
            for h in range(2):
                nc.scalar.dma_start(wqT[h][:], wqT_d[h])
                nc.sync.dma_start(xtT[h][:], xtT_d[h])
            for h in range(2):
                nc.gpsimd.dma_start(wvT[h][:], wvT_d[h])
                nc.gpsimd.dma_start(selb[h][:], selb_d[h])
            for jj in range(4):
                nc.gpsimd.dma_start(sel8[jj][:], self8_d[jj])
            nc.gpsimd.dma_start(vmask[:], vmask_d[:])
            nc.gpsimd.dma_start(ones_sel[:], ones_d[:])

            # ---- projections: K and Q first (the elementwise pipeline needs
            # them), V after ----
            QT = [cpool.tile([128, L1], BF16, name=f"QT{h}") for h in range(2)]
            KTn = [cpool.tile([128, JSH], F32, name=f"KTn{h}") for h in range(2)]
            for eh in range(2):
                psk = ps_pool.tile([128, JSH], F32, name="psk", tag="ps_s")
                for dh in range(2):
                    nc.tensor.matmul(
                        psk[:],
                        wkT[dh][:, eh * 128 : (eh + 1) * 128],
                        xssT[dh][:],
                        start=(dh == 0),
                        stop=(dh == 1),
                    )
                nc.scalar.mul(KTn[eh][:], psk[:], -1.0)
                psq = ps_pool.tile([128, L1], F32, name="psq", tag="ps_s")
                for dh in range(2):
                    nc.tensor.matmul(
                        psq[:],
                        wqT[dh][:, eh * 128 : (eh + 1) * 128],
                        xtT[dh][:],
                        start=(dh == 0),
                        stop=(dh == 1),
                    )
                nc.scalar.copy(QT[eh][:], psq[:])

            # ---- V projection -> DRAM -> per-pair broadcast+mask ----
            Vt = cpool.tile([128, D], BF16, name="Vt")
            psv = ps_pool.tile([128, D], F32, name="psv", tag="ps_s")
            for dh in range(2):
                nc.tensor.matmul(
                    psv[:],
                    xssT[dh][:],
                    wvT[dh][:],
                    start=(dh == 0),
                    stop=(dh == 1),
                )
            nc.scalar.copy(Vt[:], psv[:])
            nc.sync.dma_start(vdram[:], Vt[:])

            V_sel = [
                vpool.tile([128, 2 * D], BF16, name=f"vs{k}") for k in range(NPAIR)
            ]

            def build_pair(k):
                vs2 = V_sel[k]
                for half in range(2):
                    gq = 2 * k + half
                    bsrc = (
                        vdram.ap()[4 * gq : 4 * gq + 4, :]
                        .unsqueeze(1)
                        .broadcast_to((4, 32, D))
                    )
                    eng = nc.sync if half == 0 else nc.gpsimd
                    eng.dma_start(vs2[:, half * D : (half + 1) * D], bsrc)
                nc.vector.tensor_tensor(vs2[:], vs2[:], vmask[:], op=AL.mult)

            for k in range(4):
                build_pair(k)

            # ---- main loop over 16 quad-pairs, software-pipelined ----
            state = {}

            def issue_units(q, half, jjs):
                g = 2 * q + half
                psp = state[q]["ps"]
                s_jjs = _s_jjs(g)
                for jj in jjs:
                    j = 4 * g + jj
                    last_mm = jj == 3
                    if jj in s_jjs:
                        # ScalarE path: fp8 t + one DoubleRow matmul
                        t = tpool.tile([128, 2, 512], FP8, name="t8", tag="t8")
                        for h in range(2):
                            nc.scalar.activation(
                                t[:, h, :],
                                QT[h][:],
                                AF.Relu,
                                bias=KTn[h][:, j : j + 1],
                                scale=1.0,
                            )
                        nc.tensor.matmul(
                            psp[:, half, :],
                            sel8[jj][:],
                            t[:],
                            start=(jj == 0),
                            stop=last_mm,
                            perf_mode=DR,
                            skip_group_check=True,
                        )
                    else:
                        # VectorE path: bf16 t + two bf16 matmuls
                        t = tpool.tile([128, 2, 512], BF16, name="tb", tag="tb")
                        for h in range(2):
                            nc.vector.tensor_scalar(
                                t[:, h, :],
                                QT[h][:],
                                KTn[h][:, j : j + 1],
                                0.0,
                                AL.add,
                                AL.max,
                            )
                            nc.tensor.matmul(
                                psp[32 * jj : 32 * jj + 32, half, :],
                                selb[h][:],
                                t[:, h, :],
                                start=False,
                                stop=(last_mm and h == 1),
                                tile_position=(0, 32 * jj),
                                skip_group_check=True,
                            )

            def issue_exp(q):
                # p = exp(z/16 + b) over the pair's 2-bank PSUM span
                p = wpool.tile([128, 2, L1], BF16, name="p", tag="p", bufs=3)
                nc.scalar.activation(
                    p[:], state[q]["ps"][:], AF.Exp, bias=bml[:], scale=EXP_SCALE
                )
                state[q]["p"] = p

            def issue_pc(q):
                # pc = max(p, 1) = exp(relu(z/16 + b))
                pc = wpool.tile([128, 2, L1], BF16, name="pc", tag="pc", bufs=4)
                nc.vector.tensor_scalar(pc[:], state[q]["p"][:], 1.0, None, AL.max)
                state[q]["pc"] = pc

            def issue_vmm(q):
                pc = state[q]["pc"]
                for half in range(2):
                    g = 2 * q + half
                    for eh in range(2):
                        off = half * D + eh * 128
                        nc.tensor.matmul(
                            ops[eh][:],
                            V_sel[q][:, off : off + 128],
                            pc[:, half, :],
                            start=(g == 0),
                            stop=(g == NQUAD - 1),
                            skip_group_check=True,
                        )
                # denominator per quad half (PE writes can't cross PSUM banks)
                for half in range(2):
                    nc.tensor.matmul(
                        sps[:, half, :],
                        ones_sel[:, 0:N],
                        pc[:, half, :],
                        start=(q == 0),
                        stop=(q == NPAIR - 1),
                        skip_group_check=True,
                    )
                del state[q]

            for q in range(NPAIR):
                if q + 4 < NPAIR:
                    build_pair(q + 4)
                state[q] = {
                    "ps": ps_pool.tile([128, 2, L1], F32, name="ps", tag="ps_s")
                }
                issue_units(q, 0, (0, 1, 2))
                if q >= 1:
                    issue_exp(q - 1)
                issue_units(q, 0, (3,))
                issue_units(q, 1, (0,))
                if q >= 1:
                    issue_pc(q - 1)
                issue_units(q, 1, (1, 2))
                if q >= 1:
                    issue_vmm(q - 1)
                issue_units(q, 1, (3,))
            issue_exp(NPAIR - 1)
            issue_pc(NPAIR - 1)
            issue_vmm(NPAIR - 1)

            # ---- evacuate + store ----
            for eh in range(2):
                ou = wpool.tile([128, L1], F32, name="ou", tag="ou", bufs=2)
                if eh == 0:
                    nc.vector.tensor_copy(ou[:], ops[eh][:])
                else:
                    nc.scalar.copy(ou[:], ops[eh][:])
                nc.sync.dma_start(outp_d[eh], ou[:])
            so = wpool.tile([16, 2, L1], F32, name="so")
            nc.scalar.copy(so[:], sps[:])
            nc.sync.dma_start(souts_d[:], so[:])

    nc.compile()
    return nc


_CACHE: dict = {}


def _get_graph(b_val: float):
    key = round(float(b_val), 10)
    if key not in _CACHE:
        _CACHE[key] = _build(float(b_val))
    return _CACHE[key]


def _host_prep(x_source, x_target, Wq, Wk, Wv, w_mlp):
    """Build per-core input maps (numpy, bf16/fp8)."""
    w_full = np.tile(np.asarray(w_mlp, np.float32), D // G)  # w_full[d] = w[d%16]
    # fp8 DoubleRow sel: per-jj variant [128 part, 2 h-planes, 128 cols],
    # nonzero only at col 32*jj + group(d)
    sel8 = np.zeros((4, 128, 2, 128), np.float32)
    for jj in range(4):
        for h in range(2):
            for dl in range(128):
                d = 128 * h + dl
                sel8[jj, dl, h, 32 * jj + d // G] = SEL_SCALE * w_full[d]
    # bf16 sel (VectorE path): [2 h][128 part, 32 cols], col = group(d).
    # Same SEL_SCALE as fp8 so both paths emit z*16 (EXP_SCALE undoes it).
    selb = np.zeros((2, 128, 32), np.float32)
    for h in range(2):
        for dl in range(128):
            d = 128 * h + dl
            selb[h, dl, d // G] = SEL_SCALE * w_full[d]
    # V_sel mask: row p = 32*jj + s (s<16 valid), col e: keep if e%16 == s
    vmask = np.zeros((128, 2 * D), np.float32)
    for p in range(128):
        s = p % 32
        if s < 16:
            vmask[p, s::G] = 1.0
    # S selector: row p = 32*jj + s -> column s (s < 16)
    ones_sel = np.zeros((128, N), np.float32)
    for p in range(128):
        s = p % 32
        if s < 16:
            ones_sel[p, s] = 1.0

    def split_h(a):  # (256, X) -> (2, 128, X)
        return np.ascontiguousarray(a.reshape(2, 128, a.shape[1]))

    wq_b = split_h(SCALE_T * np.asarray(Wq, np.float32).T).astype(BF)
    wk_b = split_h(SCALE_T * np.asarray(Wk, np.float32).T).astype(BF)
    wv_b = split_h(np.asarray(Wv, np.float32).T).astype(BF)
    sel8_b = sel8.astype(F8)
    selb_b = selb.astype(BF)
    vmask_b = vmask.astype(BF)
    ones_b = ones_sel.astype(BF)

    xtT = [
        split_h(np.asarray(x_target[b], np.float32).T).astype(BF) for b in range(B)
    ]
    xsT = [np.asarray(x_source[b], np.float32).T for b in range(B)]
    in_maps = []
    for core in range(NCORES):
        b, jq = divmod(core, 4)
        j0 = jq * JSH
        xssT = split_h(xsT[b][:, j0 : j0 + JSH]).astype(BF)
        in_maps.append(
            {
                "xtT": xtT[b],
                "xssT": xssT,
                "wqT": wq_b,
                "wkT": wk_b,
                "wvT": wv_b,
                "sel8": sel8_b,
                "selb": selb_b,
                "vmask": vmask_b,
                "ones_sel": ones_b,
            }
        )
    return in_maps


def _host_gather(results):
    """Sum partials over j-shards, normalize, reshape to (B, L1, D)."""
    out = np.empty((B, L1, D), np.float32)
    for b in range(B):
        cores = [b * 4 + jq for jq in range(4)]
        U = sum(
            results[c]["outp"].reshape(D, L1).astype(np.float64) for c in cores
        )  # (e, i)
        S = sum(
            results[c]["souts"].sum(axis=1).astype(np.float64) for c in cores
        )  # (nn, i): paired denominator halves summed
        att = U / S[np.arange(D) % N, :]  # (e, i)
        out[b] = att.T.astype(np.float32)
    return out


def run(inputs, trace=False, **kwargs):
    nc = _get_graph(float(np.asarray(inputs["b_mlp"]).reshape(-1)[0]))
    in_maps = _host_prep(
        inputs["x_source"],
        inputs["x_target"],
        inputs["Wq"],
        inputs["Wk"],
        inputs["Wv"],
        inputs["w_mlp"],
    )
    res = run_bass_kernel_spmd(
        nc, in_maps, core_ids=list(range(NCORES)), trace=trace, **kwargs
    )
    return _host_gather(res.results), res


def kernel(**inputs) -> np.ndarray:
    out, _ = run(inputs, trace=False)
    return out
